# revision 15
# baseline (speedup 1.0000x reference)
"""Trainium2 Bass kernel for nn_Attention_51900384805169.

Vision-transformer attention block: qkv 1x1 conv + BN, 4-head attention
(key_dim 32, head_dim 64, N=2304 tokens), depthwise-3x3 positional branch
on v, projection 1x1 conv + BN.

Sharding: data-parallel over batch B=8 across the 8 NeuronCores (one image
per core, no collectives). BN is folded into conv weights on the host.

Per-core device pipeline (all matmuls float32r):
  1. qkv matmul with output channels permuted into [q_all | k_all | v0 | v1]
     blocks so each head h's q/k live at partitions 32h..32h+32 (enables
     per-head PE row-group placement for the K=32 score matmuls).
  2. v^T computed directly by a transposed matmul (lhsT = x tile), augmented
     with a ones column so the attention-output matmul also produces the
     softmax denominator.
  3. Scores computed transposed (S^T = k^T q: keys on partitions, queries on
     free axis); exp via ScalarE with the 1/sqrt(kd) scale folded into the
     activation's free affine. No max-subtraction (|scores| <= ~10).
  4. out_unnorm[{d,1}, q] = v_aug @ P^T accumulated over key chunks in PSUM;
     row 64 is the softmax denominator. Normalize with DVE reciprocal +
     GpSimd partition-broadcast + DVE multiply.
  5. Depthwise 3x3 via 9 fused scalar_tensor_tensor ops on DVE.
  6. proj matmul + bias, DMA out.
"""

import sys

if "/opt/trn_rl_repo" not in sys.path:
    sys.path.insert(0, "/opt/trn_rl_repo")

import numpy as np

C = 256
NH = 4
KD = 32
HD = 64
N = 2304
HW = 48
SCALE = KD ** -0.5
BN_EPS = 1e-3

# n-dim chunking for 512-wide matmuls (f32r needs free >= 256 for full rate)
NSUBS = [(0, 512), (512, 512), (1024, 512), (1536, 512), (2048, 256)]
# query chunks for the attention loop (PSUM: [128,1024] = 2 banks)
QCS = [(0, 1024), (1024, 1024), (2048, 256)]
NKC = N // 128  # 18 key chunks


def _build_module():
    import concourse.bass as bass
    import concourse.tile as tile
    from concourse import bacc, mybir

    f32 = mybir.dt.float32
    f32r = mybir.dt.float32r
    AF = mybir.ActivationFunctionType
    OP = mybir.AluOpType

    nc = bacc.Bacc(None, target_bir_lowering=False)

    x_d = nc.declare_dram_parameter("x", [C, N], f32r, isOutput=False)
    wqkvT_d = nc.declare_dram_parameter("wqkvT", [C, 2 * C], f32r, isOutput=False)
    bqkv_d = nc.declare_dram_parameter("bqkv", [128, 4], f32, isOutput=False)
    bv_d = nc.declare_dram_parameter("bv", [1, C], f32, isOutput=False)
    wprojT_d = nc.declare_dram_parameter("wprojT", [C, C], f32r, isOutput=False)
    bproj_d = nc.declare_dram_parameter("bproj", [128, 2], f32, isOutput=False)
    dww_d = nc.declare_dram_parameter("dww", [128, 2, 9], f32, isOutput=False)
    bpe_d = nc.declare_dram_parameter("bpe", [128, 2], f32, isOutput=False)
    out_d = nc.declare_dram_parameter("out", [C, N], f32, isOutput=True)
    # scratch for the softmax-denominator partition-broadcast bounce
    lb_d = nc.dram_tensor("lbounce", [NH * 3, 1024], f32)

    from contextlib import ExitStack

    with tile.TileContext(nc) as tc, ExitStack() as ctx:
        sb = ctx.enter_context(tc.tile_pool(name="sb", bufs=1))
        pT_pool = ctx.enter_context(tc.tile_pool(name="pT", bufs=3))
        small = ctx.enter_context(tc.tile_pool(name="small", bufs=2))
        ps_S = ctx.enter_context(tc.tile_pool(name="ps_S", bufs=2, space="PSUM"))
        ps_mm = ctx.enter_context(tc.tile_pool(name="ps_mm", bufs=2, space="PSUM"))

        # ---- load inputs ----
        x_sb = sb.tile([128, 2, N], f32r, tag="x")
        nc.sync.dma_start(out=x_sb[:], in_=x_d[:].rearrange("(a p) n -> p a n", p=128))

        wq_sb = sb.tile([128, 2, 2 * C], f32r, tag="wq")
        nc.sync.dma_start(
            out=wq_sb[:], in_=wqkvT_d[:].rearrange("(a p) o -> p a o", p=128)
        )
        wp_sb = sb.tile([128, 2, C], f32r, tag="wp")
        nc.sync.dma_start(
            out=wp_sb[:], in_=wprojT_d[:].rearrange("(a p) o -> p a o", p=128)
        )
        bqkv_sb = sb.tile([128, 4], f32, tag="bqkv")
        nc.sync.dma_start(out=bqkv_sb[:], in_=bqkv_d[:])
        bproj_sb = sb.tile([128, 2], f32, tag="bproj")
        nc.sync.dma_start(out=bproj_sb[:], in_=bproj_d[:])
        dww_sb = sb.tile([128, 2, 9], f32, tag="dww")
        nc.sync.dma_start(out=dww_sb[:], in_=dww_d[:])
        bpe_sb = sb.tile([128, 2], f32, tag="bpe")
        nc.sync.dma_start(out=bpe_sb[:], in_=bpe_d[:])
        bv_row = sb.tile([1, C], f32, tag="bv_row")
        nc.sync.dma_start(out=bv_row[:], in_=bv_d[:])

        # v bias broadcast across partitions (used for the free-axis bias add
        # of the transposed-v matmul)
        bv_bc = sb.tile([128, C], f32, tag="bv_bc")
        nc.sync.dma_start(out=bv_bc[:], in_=bv_d[:].to_broadcast((128, C)))

        # ---- qkv matmul (channels permuted to q_all | k_all | v0 | v1) ----
        q_all = sb.tile([128, N], f32r, tag="q_all")
        k_all = sb.tile([128, N], f32r, tag="k_all")
        v_img = sb.tile([128, 2, N], f32, tag="v_img")
        for oc in range(4):
            for n0, nw in NSUBS:
                pq = ps_mm.tile([128, 512], f32, tag="mm")
                nc.tensor.matmul(
                    pq[:, 0:nw],
                    wq_sb[:, 0, oc * 128 : (oc + 1) * 128],
                    x_sb[:, 0, n0 : n0 + nw],
                    start=True,
                    stop=False,
                )
                nc.tensor.matmul(
                    pq[:, 0:nw],
                    wq_sb[:, 1, oc * 128 : (oc + 1) * 128],
                    x_sb[:, 1, n0 : n0 + nw],
                    start=False,
                    stop=True,
                )
                if oc == 0:
                    dst = q_all[:, n0 : n0 + nw]
                elif oc == 1:
                    dst = k_all[:, n0 : n0 + nw]
                else:
                    dst = v_img[:, oc - 2, n0 : n0 + nw]
                nc.vector.tensor_scalar(
                    dst, pq[:, 0:nw], bqkv_sb[:, oc : oc + 1], None, OP.add
                )

        # ---- transposed v (v^T) with ones column ----
        # vaugT[p, kc, h, 0:64] = v[h*64+d, kc*128+p];  [.., 64] = 1.0
        vaugT = sb.tile([128, NKC, NH, HD + 1], f32r, tag="vaugT")
        # ones everywhere; the [0:64] data columns get overwritten below, so
        # only the 65th (denominator) column keeps the 1.0
        nc.vector.tensor_scalar(
            vaugT[:].rearrange("p a b c -> p (a b c)"),
            bv_bc[:, 0:1].to_broadcast((128, NKC * NH * (HD + 1))),
            0.0,
            1.0,
            OP.mult,
            OP.add,
        )
        for kc in range(NKC):
            pv = ps_mm.tile([128, 256], f32, tag="mm")
            nc.tensor.matmul(
                pv[:],
                x_sb[:, 0, kc * 128 : (kc + 1) * 128],
                wq_sb[:, 0, C : 2 * C],
                start=True,
                stop=False,
            )
            nc.tensor.matmul(
                pv[:],
                x_sb[:, 1, kc * 128 : (kc + 1) * 128],
                wq_sb[:, 1, C : 2 * C],
                start=False,
                stop=True,
            )
            nc.vector.tensor_tensor(
                out=vaugT[:, kc, :, 0:HD],
                in0=pv[:].rearrange("p (h d) -> p h d", h=NH),
                in1=bv_bc[:].rearrange("p (h d) -> p h d", h=NH),
                op=OP.add,
            )

        # ---- attention ----
        attn = sb.tile([128, 2, N], f32r, tag="attn")
        for h in range(NH):
            hp = 32 * h
            for qci, (q0, qw) in enumerate(QCS):
                pO = ps_mm.tile([HD + 1, 1024], f32, tag="mm")
                for kc in range(NKC):
                    pS = ps_S.tile([128, 1024], f32, tag="S")
                    for s0 in range(0, qw, 512):
                        sw = min(512, qw - s0)
                        nc.tensor.matmul(
                            pS[:, s0 : s0 + sw],
                            k_all[hp : hp + 32, kc * 128 : (kc + 1) * 128],
                            q_all[hp : hp + 32, q0 + s0 : q0 + s0 + sw],
                            start=True,
                            stop=True,
                            tile_position=(hp, 0),
                        )
                    pT = pT_pool.tile([128, 1024], f32r, tag="pT")
                    nc.scalar.activation(
                        out=pT[:, 0:qw], in_=pS[:, 0:qw], func=AF.Exp, scale=SCALE
                    )
                    for s0 in range(0, qw, 512):
                        sw = min(512, qw - s0)
                        nc.tensor.matmul(
                            pO[:, s0 : s0 + sw],
                            vaugT[:, kc, h, :],
                            pT[:, s0 : s0 + sw],
                            start=(kc == 0),
                            stop=(kc == NKC - 1),
                        )
                # normalize: attn[d, q] = pO[d, q] / pO[64, q]
                linv = small.tile([1, 1024], f32, tag="linv")
                nc.vector.reciprocal(out=linv[:, 0:qw], in_=pO[HD : HD + 1, 0:qw])
                lbi = h * 3 + qci
                nc.sync.dma_start(out=lb_d[lbi : lbi + 1, 0:qw], in_=linv[:, 0:qw])
                linv_bc = small.tile([64, 1024], f32, tag="linv_bc")
                nc.sync.dma_start(
                    out=linv_bc[:, 0:qw],
                    in_=lb_d[lbi : lbi + 1, 0:qw].to_broadcast((64, qw)),
                )
                r0 = (h % 2) * 64
                nc.vector.tensor_tensor(
                    out=attn[r0 : r0 + 64, h // 2, q0 : q0 + qw],
                    in0=pO[0:HD, 0:qw],
                    in1=linv_bc[:, 0:qw],
                    op=OP.mult,
                )

        # ---- depthwise 3x3 conv on v_img (BN folded into dww/bpe) ----
        pe = sb.tile([128, 2, HW, HW], f32, tag="pe")
        v3 = v_img[:].rearrange("p a (y x) -> p a y x", y=HW)
        for a in range(2):
            # center tap (ky=1, kx=1) initializes
            nc.vector.tensor_scalar(
                pe[:, a], v3[:, a], dww_sb[:, a, 4:5], None, OP.mult
            )
            for ky in range(3):
                for kx in range(3):
                    if ky == 1 and kx == 1:
                        continue
                    dy, dx = ky - 1, kx - 1
                    y0, y1 = max(0, -dy), HW - max(0, dy)
                    x0, x1 = max(0, -dx), HW - max(0, dx)
                    t = ky * 3 + kx
                    nc.vector.scalar_tensor_tensor(
                        out=pe[:, a, y0:y1, x0:x1],
                        in0=v3[:, a, y0 + dy : y1 + dy, x0 + dx : x1 + dx],
                        scalar=dww_sb[:, a, t : t + 1],
                        in1=pe[:, a, y0:y1, x0:x1],
                        op0=OP.mult,
                        op1=OP.add,
                    )
            # attn += pe + bpe  (fused: (pe add bpe) add attn)
            nc.vector.scalar_tensor_tensor(
                out=attn[:, a, :],
                in0=pe[:, a].rearrange("p y x -> p (y x)"),
                scalar=bpe_sb[:, a : a + 1],
                in1=attn[:, a, :],
                op0=OP.add,
                op1=OP.add,
            )

        # ---- proj matmul ----
        out_sb = sb.tile([128, 2, N], f32, tag="out_sb")
        for oc in range(2):
            for n0, nw in NSUBS:
                pp = ps_mm.tile([128, 512], f32, tag="mm")
                nc.tensor.matmul(
                    pp[:, 0:nw],
                    wp_sb[:, 0, oc * 128 : (oc + 1) * 128],
                    attn[:, 0, n0 : n0 + nw],
                    start=True,
                    stop=False,
                )
                nc.tensor.matmul(
                    pp[:, 0:nw],
                    wp_sb[:, 1, oc * 128 : (oc + 1) * 128],
                    attn[:, 1, n0 : n0 + nw],
                    start=False,
                    stop=True,
                )
                nc.vector.tensor_scalar(
                    out_sb[:, oc, n0 : n0 + nw],
                    pp[:, 0:nw],
                    bproj_sb[:, oc : oc + 1],
                    None,
                    OP.add,
                )

        nc.sync.dma_start(
            out=out_d[:].rearrange("(a p) n -> p a n", p=128), in_=out_sb[:]
        )

    nc.finalize()
    return nc


def _prep_weights(qkv_w, qkv_g, qkv_b, qkv_m, qkv_v,
                  pe_w, pe_g, pe_b, pe_m, pe_v,
                  proj_w, proj_g, proj_b, proj_m, proj_v):
    """Fold BN into conv weights (host-side, numpy) and permute qkv output
    channels into [q_all | k_all | v0 | v1] blocks."""
    f = np.float32

    inv_q = (qkv_g / np.sqrt(qkv_v + BN_EPS)).astype(f)
    Wq = (qkv_w * inv_q[:, None]).astype(f)
    bq = (qkv_b - qkv_m * inv_q).astype(f)

    perm = []
    for h in range(NH):
        perm += list(range(h * 128, h * 128 + 32))          # q
    for h in range(NH):
        perm += list(range(h * 128 + 32, h * 128 + 64))     # k
    for h in range(NH):
        perm += list(range(h * 128 + 64, h * 128 + 128))    # v
    perm = np.array(perm)

    Wq_p = Wq[perm]            # [512, 256]
    bq_p = bq[perm]            # [512]
    wqkvT = np.ascontiguousarray(Wq_p.T)                    # [256, 512]
    bqkv = np.ascontiguousarray(bq_p.reshape(4, 128).T)     # [128, 4]
    bv = np.ascontiguousarray(bq_p[256:512].reshape(1, C))  # [1, 256]

    inv_pe = (pe_g / np.sqrt(pe_v + BN_EPS)).astype(f)
    dw = (pe_w[:, 0] * inv_pe[:, None, None]).astype(f).reshape(C, 9)
    dww = np.ascontiguousarray(dw.reshape(2, 128, 9).transpose(1, 0, 2))
    bpe = np.ascontiguousarray(
        (pe_b - pe_m * inv_pe).astype(f).reshape(2, 128).T
    )

    inv_p = (proj_g / np.sqrt(proj_v + BN_EPS)).astype(f)
    Wp = (proj_w * inv_p[:, None]).astype(f)
    bp = (proj_b - proj_m * inv_p).astype(f)
    wprojT = np.ascontiguousarray(Wp.T)                     # [256, 256]
    bproj = np.ascontiguousarray(bp.reshape(2, 128).T)      # [128, 2]

    return dict(wqkvT=wqkvT, bqkv=bqkv, bv=bv, wprojT=wprojT, bproj=bproj,
                dww=dww, bpe=bpe)


_NC_CACHE = None


def _get_module():
    global _NC_CACHE
    if _NC_CACHE is None:
        _NC_CACHE = _build_module()
    return _NC_CACHE


def kernel(x, qkv_w, qkv_g, qkv_b, qkv_m, qkv_v,
           pe_w, pe_g, pe_b, pe_m, pe_v,
           proj_w, proj_g, proj_b, proj_m, proj_v,
           _trace=False):
    from concourse.bass_utils import run_bass_kernel_spmd

    B = x.shape[0]
    assert x.shape == (B, C, HW, HW)
    w = _prep_weights(qkv_w, qkv_g, qkv_b, qkv_m, qkv_v,
                      pe_w, pe_g, pe_b, pe_m, pe_v,
                      proj_w, proj_g, proj_b, proj_m, proj_v)

    nc = _get_module()
    in_maps = []
    for b in range(B):
        m = {"x": np.ascontiguousarray(x[b].reshape(C, N), np.float32)}
        m.update(w)
        in_maps.append(m)

    res = run_bass_kernel_spmd(nc, in_maps, core_ids=list(range(B)),
                               trace=_trace)
    out = np.stack([res.results[b]["out"] for b in range(B)])
    out = out.reshape(B, C, HW, HW)
    if _trace:
        return out, res
    return out


# revision 17
# speedup vs baseline: 1.0257x; 1.0257x over previous
"""Trainium2 Bass kernel for nn_Attention_51900384805169.

Vision-transformer attention block: qkv 1x1 conv + BN, 4-head attention
(key_dim 32, head_dim 64, N=2304 tokens), depthwise-3x3 positional branch
on v, projection 1x1 conv + BN.

Sharding: data-parallel over batch B=8 across the 8 NeuronCores (one image
per core, no collectives). BN is folded into conv weights on the host.

Per-core device pipeline (all matmuls float32r):
  1. qkv matmul with output channels permuted into [q_all | k_all | v0 | v1]
     blocks so each head h's q/k live at partitions 32h..32h+32 (enables
     per-head PE row-group placement for the K=32 score matmuls).
  2. v^T computed directly by a transposed matmul (lhsT = x tile), augmented
     with a ones column so the attention-output matmul also produces the
     softmax denominator.
  3. Scores computed transposed (S^T = k^T q: keys on partitions, queries on
     free axis); exp via ScalarE with the 1/sqrt(kd) scale folded into the
     activation's free affine. No max-subtraction (|scores| <= ~10).
  4. out_unnorm[{d,1}, q] = v_aug @ P^T accumulated over key chunks in PSUM;
     row 64 is the softmax denominator. Normalize with DVE reciprocal +
     GpSimd partition-broadcast + DVE multiply.
  5. Depthwise 3x3 via 9 fused scalar_tensor_tensor ops on DVE.
  6. proj matmul + bias, DMA out.
"""

import sys

if "/opt/trn_rl_repo" not in sys.path:
    sys.path.insert(0, "/opt/trn_rl_repo")

import numpy as np

C = 256
NH = 4
KD = 32
HD = 64
N = 2304
HW = 48
SCALE = KD ** -0.5
BN_EPS = 1e-3

# n-dim chunking for 512-wide matmuls (f32r needs free >= 256 for full rate)
NSUBS = [(0, 512), (512, 512), (1024, 512), (1536, 512), (2048, 256)]
# query chunks for the attention loop (PSUM: [128,1024] = 2 banks)
QCS = [(0, 1024), (1024, 1024), (2048, 256)]
NKC = N // 128  # 18 key chunks


def _build_module():
    import concourse.bass as bass
    import concourse.tile as tile
    from concourse import bacc, mybir

    f32 = mybir.dt.float32
    f32r = mybir.dt.float32r
    AF = mybir.ActivationFunctionType
    OP = mybir.AluOpType

    nc = bacc.Bacc(None, target_bir_lowering=False)

    x_d = nc.declare_dram_parameter("x", [C, N], f32r, isOutput=False)
    wqkvT_d = nc.declare_dram_parameter("wqkvT", [C, 2 * C], f32r, isOutput=False)
    bqkv_d = nc.declare_dram_parameter("bqkv", [128, 4], f32, isOutput=False)
    bv_d = nc.declare_dram_parameter("bv", [1, C], f32, isOutput=False)
    wprojT_d = nc.declare_dram_parameter("wprojT", [C, C], f32r, isOutput=False)
    bproj_d = nc.declare_dram_parameter("bproj", [128, 2], f32, isOutput=False)
    dww_d = nc.declare_dram_parameter("dww", [128, 2, 9], f32, isOutput=False)
    bpe_d = nc.declare_dram_parameter("bpe", [128, 2], f32, isOutput=False)
    out_d = nc.declare_dram_parameter("out", [C, N], f32, isOutput=True)
    # scratch for the softmax-denominator partition-broadcast bounce
    lb_d = nc.dram_tensor("lbounce", [NH * 3, 1024], f32)
    rb_d = nc.dram_tensor("rbounce", [NH * 3, 1024], f32)

    from contextlib import ExitStack

    with tile.TileContext(nc) as tc, ExitStack() as ctx:
        sb = ctx.enter_context(tc.tile_pool(name="sb", bufs=1))
        pT_pool = ctx.enter_context(tc.tile_pool(name="pT", bufs=3))
        small = ctx.enter_context(tc.tile_pool(name="small", bufs=2))
        ps_S = ctx.enter_context(tc.tile_pool(name="ps_S", bufs=2, space="PSUM"))
        ps_mm = ctx.enter_context(tc.tile_pool(name="ps_mm", bufs=2, space="PSUM"))

        # ---- load inputs ----
        x_sb = sb.tile([128, 2, N], f32r, tag="x")
        nc.sync.dma_start(out=x_sb[:], in_=x_d[:].rearrange("(a p) n -> p a n", p=128))

        wq_sb = sb.tile([128, 2, 2 * C], f32r, tag="wq")
        nc.sync.dma_start(
            out=wq_sb[:], in_=wqkvT_d[:].rearrange("(a p) o -> p a o", p=128)
        )
        wp_sb = sb.tile([128, 2, C], f32r, tag="wp")
        nc.sync.dma_start(
            out=wp_sb[:], in_=wprojT_d[:].rearrange("(a p) o -> p a o", p=128)
        )
        bqkv_sb = sb.tile([128, 4], f32, tag="bqkv")
        nc.sync.dma_start(out=bqkv_sb[:], in_=bqkv_d[:])
        bproj_sb = sb.tile([128, 2], f32, tag="bproj")
        nc.sync.dma_start(out=bproj_sb[:], in_=bproj_d[:])
        dww_sb = sb.tile([128, 2, 9], f32, tag="dww")
        nc.sync.dma_start(out=dww_sb[:], in_=dww_d[:])
        bpe_sb = sb.tile([128, 2], f32, tag="bpe")
        nc.sync.dma_start(out=bpe_sb[:], in_=bpe_d[:])
        bv_row = sb.tile([1, C], f32, tag="bv_row")
        nc.sync.dma_start(out=bv_row[:], in_=bv_d[:])

        # v bias broadcast across partitions (used for the free-axis bias add
        # of the transposed-v matmul)
        bv_bc = sb.tile([128, C], f32, tag="bv_bc")
        nc.sync.dma_start(out=bv_bc[:], in_=bv_d[:].to_broadcast((128, C)))

        # ---- qkv matmul (channels permuted to q_all | k_all | v0 | v1) ----
        q_all = sb.tile([128, N], f32r, tag="q_all")
        k_all = sb.tile([128, N], f32r, tag="k_all")
        v_img = sb.tile([128, 2, N], f32, tag="v_img")
        for oc in range(4):
            for n0, nw in NSUBS:
                pq = ps_mm.tile([128, 512], f32, tag="mm")
                nc.tensor.matmul(
                    pq[:, 0:nw],
                    wq_sb[:, 0, oc * 128 : (oc + 1) * 128],
                    x_sb[:, 0, n0 : n0 + nw],
                    start=True,
                    stop=False,
                )
                nc.tensor.matmul(
                    pq[:, 0:nw],
                    wq_sb[:, 1, oc * 128 : (oc + 1) * 128],
                    x_sb[:, 1, n0 : n0 + nw],
                    start=False,
                    stop=True,
                )
                if oc == 0:
                    dst = q_all[:, n0 : n0 + nw]
                elif oc == 1:
                    dst = k_all[:, n0 : n0 + nw]
                else:
                    dst = v_img[:, oc - 2, n0 : n0 + nw]
                nc.vector.tensor_scalar(
                    dst, pq[:, 0:nw], bqkv_sb[:, oc : oc + 1], None, OP.add
                )

        # ---- transposed v (v^T) with ones column ----
        # vaugT[p, kc, h, 0:64] = v[h*64+d, kc*128+p];  [.., 64] = 1.0
        vaugT = sb.tile([128, NKC, NH, HD + 1], f32r, tag="vaugT")
        # ones everywhere; the [0:64] data columns get overwritten below, so
        # only the 65th (denominator) column keeps the 1.0
        nc.vector.tensor_scalar(
            vaugT[:].rearrange("p a b c -> p (a b c)"),
            bv_bc[:, 0:1].to_broadcast((128, NKC * NH * (HD + 1))),
            0.0,
            1.0,
            OP.mult,
            OP.add,
        )
        for kc in range(NKC):
            pv = ps_mm.tile([128, 256], f32, tag="mm")
            nc.tensor.matmul(
                pv[:],
                x_sb[:, 0, kc * 128 : (kc + 1) * 128],
                wq_sb[:, 0, C : 2 * C],
                start=True,
                stop=False,
            )
            nc.tensor.matmul(
                pv[:],
                x_sb[:, 1, kc * 128 : (kc + 1) * 128],
                wq_sb[:, 1, C : 2 * C],
                start=False,
                stop=True,
            )
            nc.vector.tensor_tensor(
                out=vaugT[:, kc, :, 0:HD],
                in0=pv[:].rearrange("p (h d) -> p h d", h=NH),
                in1=bv_bc[:].rearrange("p (h d) -> p h d", h=NH),
                op=OP.add,
            )

        # ---- attention ----
        attn = sb.tile([128, 2, N], f32r, tag="attn")
        for h in range(NH):
            hp = 32 * h
            for qci, (q0, qw) in enumerate(QCS):
                pO = ps_mm.tile([HD + 1, 1024], f32, tag="mm")

                def score_stage(kc):
                    pS = ps_S.tile([128, 1024], f32, tag="S")
                    for s0 in range(0, qw, 512):
                        sw = min(512, qw - s0)
                        nc.tensor.matmul(
                            pS[:, s0 : s0 + sw],
                            k_all[hp : hp + 32, kc * 128 : (kc + 1) * 128],
                            q_all[hp : hp + 32, q0 + s0 : q0 + s0 + sw],
                            start=True,
                            stop=True,
                            tile_position=(hp, 0),
                        )
                    pT = pT_pool.tile([128, 1024], f32r, tag="pT")
                    nc.scalar.activation(
                        out=pT[:, 0:qw], in_=pS[:, 0:qw], func=AF.Exp, scale=SCALE
                    )
                    return pT

                def out_stage(kc, pT):
                    for s0 in range(0, qw, 512):
                        sw = min(512, qw - s0)
                        nc.tensor.matmul(
                            pO[:, s0 : s0 + sw],
                            vaugT[:, kc, h, :],
                            pT[:, s0 : s0 + sw],
                            start=(kc == 0),
                            stop=(kc == NKC - 1),
                        )

                # software pipeline: issue S^T(kc+1) before out(kc) so the PE
                # never idles waiting for exp(kc)
                pT_prev = score_stage(0)
                for kc in range(1, NKC):
                    pT_cur = score_stage(kc)
                    out_stage(kc - 1, pT_prev)
                    pT_prev = pT_cur
                out_stage(NKC - 1, pT_prev)

                # normalize: attn[d, q] = pO[d, q] / pO[64, q].  The softmax
                # denominator row is bounced through DRAM twice: once to
                # reshape [1, qw] -> [128, qw/128] so the reciprocal runs on
                # all 128 lanes, once to broadcast 1/l across 64 partitions.
                lbi = h * 3 + qci
                lrow = small.tile([1, 1024], f32, tag="lrow")
                nc.vector.tensor_copy(lrow[:, 0:qw], pO[HD : HD + 1, 0:qw])
                nc.sync.dma_start(out=lb_d[lbi : lbi + 1, 0:qw], in_=lrow[:, 0:qw])
                lpar = small.tile([128, 8], f32, tag="lpar")
                fw = qw // 128
                nc.sync.dma_start(
                    out=lpar[:, 0:fw],
                    in_=lb_d[lbi, 0:qw].rearrange("(p f) -> p f", p=128),
                )
                rpar = small.tile([128, 8], f32, tag="rpar")
                nc.vector.reciprocal(out=rpar[:, 0:fw], in_=lpar[:, 0:fw])
                nc.sync.dma_start(
                    out=rb_d[lbi, 0:qw].rearrange("(p f) -> p f", p=128),
                    in_=rpar[:, 0:fw],
                )
                linv_bc = small.tile([64, 1024], f32, tag="linv_bc")
                nc.sync.dma_start(
                    out=linv_bc[:, 0:qw],
                    in_=rb_d[lbi : lbi + 1, 0:qw].to_broadcast((64, qw)),
                )
                r0 = (h % 2) * 64
                nc.vector.tensor_tensor(
                    out=attn[r0 : r0 + 64, h // 2, q0 : q0 + qw],
                    in0=pO[0:HD, 0:qw],
                    in1=linv_bc[:, 0:qw],
                    op=OP.mult,
                )

        # ---- depthwise 3x3 conv on v_img (BN folded into dww/bpe) ----
        pe = sb.tile([128, 2, HW, HW], f32, tag="pe")
        v3 = v_img[:].rearrange("p a (y x) -> p a y x", y=HW)
        for a in range(2):
            # center tap (ky=1, kx=1) initializes
            nc.vector.tensor_scalar(
                pe[:, a], v3[:, a], dww_sb[:, a, 4:5], None, OP.mult
            )
            for ky in range(3):
                for kx in range(3):
                    if ky == 1 and kx == 1:
                        continue
                    dy, dx = ky - 1, kx - 1
                    y0, y1 = max(0, -dy), HW - max(0, dy)
                    x0, x1 = max(0, -dx), HW - max(0, dx)
                    t = ky * 3 + kx
                    nc.vector.scalar_tensor_tensor(
                        out=pe[:, a, y0:y1, x0:x1],
                        in0=v3[:, a, y0 + dy : y1 + dy, x0 + dx : x1 + dx],
                        scalar=dww_sb[:, a, t : t + 1],
                        in1=pe[:, a, y0:y1, x0:x1],
                        op0=OP.mult,
                        op1=OP.add,
                    )
            # attn += pe + bpe  (fused: (pe add bpe) add attn)
            nc.vector.scalar_tensor_tensor(
                out=attn[:, a, :],
                in0=pe[:, a].rearrange("p y x -> p (y x)"),
                scalar=bpe_sb[:, a : a + 1],
                in1=attn[:, a, :],
                op0=OP.add,
                op1=OP.add,
            )

        # ---- proj matmul ----
        out_sb = sb.tile([128, 2, N], f32, tag="out_sb")
        for oc in range(2):
            for n0, nw in NSUBS:
                pp = ps_mm.tile([128, 512], f32, tag="mm")
                nc.tensor.matmul(
                    pp[:, 0:nw],
                    wp_sb[:, 0, oc * 128 : (oc + 1) * 128],
                    attn[:, 0, n0 : n0 + nw],
                    start=True,
                    stop=False,
                )
                nc.tensor.matmul(
                    pp[:, 0:nw],
                    wp_sb[:, 1, oc * 128 : (oc + 1) * 128],
                    attn[:, 1, n0 : n0 + nw],
                    start=False,
                    stop=True,
                )
                nc.vector.tensor_scalar(
                    out_sb[:, oc, n0 : n0 + nw],
                    pp[:, 0:nw],
                    bproj_sb[:, oc : oc + 1],
                    None,
                    OP.add,
                )

        nc.sync.dma_start(
            out=out_d[:].rearrange("(a p) n -> p a n", p=128), in_=out_sb[:]
        )

    nc.finalize()
    return nc


def _prep_weights(qkv_w, qkv_g, qkv_b, qkv_m, qkv_v,
                  pe_w, pe_g, pe_b, pe_m, pe_v,
                  proj_w, proj_g, proj_b, proj_m, proj_v):
    """Fold BN into conv weights (host-side, numpy) and permute qkv output
    channels into [q_all | k_all | v0 | v1] blocks."""
    f = np.float32

    inv_q = (qkv_g / np.sqrt(qkv_v + BN_EPS)).astype(f)
    Wq = (qkv_w * inv_q[:, None]).astype(f)
    bq = (qkv_b - qkv_m * inv_q).astype(f)

    perm = []
    for h in range(NH):
        perm += list(range(h * 128, h * 128 + 32))          # q
    for h in range(NH):
        perm += list(range(h * 128 + 32, h * 128 + 64))     # k
    for h in range(NH):
        perm += list(range(h * 128 + 64, h * 128 + 128))    # v
    perm = np.array(perm)

    Wq_p = Wq[perm]            # [512, 256]
    bq_p = bq[perm]            # [512]
    wqkvT = np.ascontiguousarray(Wq_p.T)                    # [256, 512]
    bqkv = np.ascontiguousarray(bq_p.reshape(4, 128).T)     # [128, 4]
    bv = np.ascontiguousarray(bq_p[256:512].reshape(1, C))  # [1, 256]

    inv_pe = (pe_g / np.sqrt(pe_v + BN_EPS)).astype(f)
    dw = (pe_w[:, 0] * inv_pe[:, None, None]).astype(f).reshape(C, 9)
    dww = np.ascontiguousarray(dw.reshape(2, 128, 9).transpose(1, 0, 2))
    bpe = np.ascontiguousarray(
        (pe_b - pe_m * inv_pe).astype(f).reshape(2, 128).T
    )

    inv_p = (proj_g / np.sqrt(proj_v + BN_EPS)).astype(f)
    Wp = (proj_w * inv_p[:, None]).astype(f)
    bp = (proj_b - proj_m * inv_p).astype(f)
    wprojT = np.ascontiguousarray(Wp.T)                     # [256, 256]
    bproj = np.ascontiguousarray(bp.reshape(2, 128).T)      # [128, 2]

    return dict(wqkvT=wqkvT, bqkv=bqkv, bv=bv, wprojT=wprojT, bproj=bproj,
                dww=dww, bpe=bpe)


_NC_CACHE = None


def _get_module():
    global _NC_CACHE
    if _NC_CACHE is None:
        _NC_CACHE = _build_module()
    return _NC_CACHE


def kernel(x, qkv_w, qkv_g, qkv_b, qkv_m, qkv_v,
           pe_w, pe_g, pe_b, pe_m, pe_v,
           proj_w, proj_g, proj_b, proj_m, proj_v,
           _trace=False):
    from concourse.bass_utils import run_bass_kernel_spmd

    B = x.shape[0]
    assert x.shape == (B, C, HW, HW)
    w = _prep_weights(qkv_w, qkv_g, qkv_b, qkv_m, qkv_v,
                      pe_w, pe_g, pe_b, pe_m, pe_v,
                      proj_w, proj_g, proj_b, proj_m, proj_v)

    nc = _get_module()
    in_maps = []
    for b in range(B):
        m = {"x": np.ascontiguousarray(x[b].reshape(C, N), np.float32)}
        m.update(w)
        in_maps.append(m)

    res = run_bass_kernel_spmd(nc, in_maps, core_ids=list(range(B)),
                               trace=_trace)
    out = np.stack([res.results[b]["out"] for b in range(B)])
    out = out.reshape(B, C, HW, HW)
    if _trace:
        return out, res
    return out


# revision 18
# speedup vs baseline: 1.4538x; 1.4173x over previous
"""Trainium2 Bass kernel for nn_Attention_51900384805169.

Vision-transformer attention block: qkv 1x1 conv + BN, 4-head attention
(key_dim 32, head_dim 64, N=2304 tokens), depthwise-3x3 positional branch
on v, projection 1x1 conv + BN.

Sharding: data-parallel over batch B=8 across the 8 NeuronCores (one image
per core, no collectives). BN is folded into conv weights on the host.

Per-core device pipeline (all matmuls float32r):
  1. qkv matmul with output channels permuted into [q_all | k_all | v0 | v1]
     blocks so each head h's q/k live at partitions 32h..32h+32 (enables
     per-head PE row-group placement for the K=32 score matmuls).
  2. v^T computed directly by a transposed matmul (lhsT = x tile), augmented
     with a ones column so the attention-output matmul also produces the
     softmax denominator.
  3. Scores computed transposed (S^T = k^T q: keys on partitions, queries on
     free axis); exp via ScalarE with the 1/sqrt(kd) scale folded into the
     activation's free affine. No max-subtraction (|scores| <= ~10).
  4. out_unnorm[{d,1}, q] = v_aug @ P^T accumulated over key chunks in PSUM;
     row 64 is the softmax denominator. Normalize with DVE reciprocal +
     GpSimd partition-broadcast + DVE multiply.
  5. Depthwise 3x3 via 9 fused scalar_tensor_tensor ops on DVE.
  6. proj matmul + bias, DMA out.
"""

import sys

if "/opt/trn_rl_repo" not in sys.path:
    sys.path.insert(0, "/opt/trn_rl_repo")

import numpy as np

C = 256
NH = 4
KD = 32
HD = 64
N = 2304
HW = 48
SCALE = KD ** -0.5
BN_EPS = 1e-3

# n-dim chunking for 512-wide matmuls (f32r needs free >= 256 for full rate)
NSUBS = [(0, 512), (512, 512), (1024, 512), (1536, 512), (2048, 256)]
# query chunks for the attention loop (PSUM: [128,1024] = 2 banks)
QCS = [(0, 1024), (1024, 1024), (2048, 256)]
NKC = N // 128  # 18 key chunks


def _build_module():
    import concourse.bass as bass
    import concourse.tile as tile
    from concourse import bacc, mybir

    f32 = mybir.dt.float32
    f32r = mybir.dt.float32r
    bf16 = mybir.dt.bfloat16
    AF = mybir.ActivationFunctionType
    OP = mybir.AluOpType

    nc = bacc.Bacc(None, target_bir_lowering=False)

    x_d = nc.declare_dram_parameter("x", [C, N], f32r, isOutput=False)
    wqkvT_d = nc.declare_dram_parameter("wqkvT", [C, 2 * C], f32r, isOutput=False)
    bqkv_d = nc.declare_dram_parameter("bqkv", [128, 4], f32, isOutput=False)
    bv_d = nc.declare_dram_parameter("bv", [1, C], f32, isOutput=False)
    wprojT_d = nc.declare_dram_parameter("wprojT", [C, C], f32r, isOutput=False)
    bproj_d = nc.declare_dram_parameter("bproj", [128, 2], f32, isOutput=False)
    dww_d = nc.declare_dram_parameter("dww", [128, 2, 9], f32, isOutput=False)
    bpe_d = nc.declare_dram_parameter("bpe", [128, 2], f32, isOutput=False)
    out_d = nc.declare_dram_parameter("out", [C, N], f32, isOutput=True)
    # scratch for the softmax-denominator partition-broadcast bounce
    lb_d = nc.dram_tensor("lbounce", [NH * 3, 1024], f32)
    rb_d = nc.dram_tensor("rbounce", [NH * 3, 1024], f32)

    from contextlib import ExitStack

    with tile.TileContext(nc) as tc, ExitStack() as ctx:
        sb = ctx.enter_context(tc.tile_pool(name="sb", bufs=1))
        pT_pool = ctx.enter_context(tc.tile_pool(name="pT", bufs=3))
        small = ctx.enter_context(tc.tile_pool(name="small", bufs=2))
        ps_S = ctx.enter_context(tc.tile_pool(name="ps_S", bufs=2, space="PSUM"))
        ps_mm = ctx.enter_context(tc.tile_pool(name="ps_mm", bufs=2, space="PSUM"))

        # ---- load inputs ----
        x_sb = sb.tile([128, 2, N], f32r, tag="x")
        nc.sync.dma_start(out=x_sb[:], in_=x_d[:].rearrange("(a p) n -> p a n", p=128))

        wq_sb = sb.tile([128, 2, 2 * C], f32r, tag="wq")
        nc.sync.dma_start(
            out=wq_sb[:], in_=wqkvT_d[:].rearrange("(a p) o -> p a o", p=128)
        )
        wp_sb = sb.tile([128, 2, C], f32r, tag="wp")
        nc.sync.dma_start(
            out=wp_sb[:], in_=wprojT_d[:].rearrange("(a p) o -> p a o", p=128)
        )
        bqkv_sb = sb.tile([128, 4], f32, tag="bqkv")
        nc.sync.dma_start(out=bqkv_sb[:], in_=bqkv_d[:])
        bproj_sb = sb.tile([128, 2], f32, tag="bproj")
        nc.sync.dma_start(out=bproj_sb[:], in_=bproj_d[:])
        dww_sb = sb.tile([128, 2, 9], f32, tag="dww")
        nc.sync.dma_start(out=dww_sb[:], in_=dww_d[:])
        bpe_sb = sb.tile([128, 2], f32, tag="bpe")
        nc.sync.dma_start(out=bpe_sb[:], in_=bpe_d[:])
        bv_row = sb.tile([1, C], f32, tag="bv_row")
        nc.sync.dma_start(out=bv_row[:], in_=bv_d[:])

        # v bias broadcast across partitions (used for the free-axis bias add
        # of the transposed-v matmul)
        bv_bc = sb.tile([128, C], f32, tag="bv_bc")
        nc.sync.dma_start(out=bv_bc[:], in_=bv_d[:].to_broadcast((128, C)))

        # ---- qkv matmul (channels permuted to q_all | k_pad | v0 | v1) ----
        # k is written into 4 zero-padded per-head tiles: k_pad[:, h, :] has
        # head h's k at partitions 32h..32h+32 and zeros elsewhere, so the
        # score matmul can contract over the full 128 partitions (keeps the
        # PE HAM activity monitor warm; K=32 row-tiled matmuls do not count
        # as activity and drop the PE clock to 1.2 GHz).
        q_all = sb.tile([128, N], bf16, tag="q_all")
        k_pad = sb.tile([128, NH, N], bf16, tag="k_pad")
        nc.vector.memset(k_pad[:], 0.0)
        v_img = sb.tile([128, 2, N], f32, tag="v_img")
        for oc in range(4):
            for n0, nw in NSUBS:
                pq = ps_mm.tile([128, 512], f32, tag="mm")
                nc.tensor.matmul(
                    pq[:, 0:nw],
                    wq_sb[:, 0, oc * 128 : (oc + 1) * 128],
                    x_sb[:, 0, n0 : n0 + nw],
                    start=True,
                    stop=False,
                )
                nc.tensor.matmul(
                    pq[:, 0:nw],
                    wq_sb[:, 1, oc * 128 : (oc + 1) * 128],
                    x_sb[:, 1, n0 : n0 + nw],
                    start=False,
                    stop=True,
                )
                if oc == 1:
                    for h in range(NH):
                        hp = 32 * h
                        nc.vector.tensor_scalar(
                            k_pad[hp : hp + 32, h, n0 : n0 + nw],
                            pq[hp : hp + 32, 0:nw],
                            bqkv_sb[hp : hp + 32, 1:2],
                            None,
                            OP.add,
                        )
                    continue
                if oc == 0:
                    dst = q_all[:, n0 : n0 + nw]
                else:
                    dst = v_img[:, oc - 2, n0 : n0 + nw]
                nc.vector.tensor_scalar(
                    dst, pq[:, 0:nw], bqkv_sb[:, oc : oc + 1], None, OP.add
                )

        # ---- transposed v (v^T) with ones column ----
        # vaugT[p, kc, h, 0:64] = v[h*64+d, kc*128+p];  [.., 64] = 1.0
        vaugT = sb.tile([128, NKC, NH, HD + 1], bf16, tag="vaugT")
        # ones everywhere; the [0:64] data columns get overwritten below, so
        # only the 65th (denominator) column keeps the 1.0
        nc.vector.tensor_scalar(
            vaugT[:].rearrange("p a b c -> p (a b c)"),
            bv_bc[:, 0:1].to_broadcast((128, NKC * NH * (HD + 1))),
            0.0,
            1.0,
            OP.mult,
            OP.add,
        )
        for kc in range(NKC):
            pv = ps_mm.tile([128, 256], f32, tag="mm")
            nc.tensor.matmul(
                pv[:],
                x_sb[:, 0, kc * 128 : (kc + 1) * 128],
                wq_sb[:, 0, C : 2 * C],
                start=True,
                stop=False,
            )
            nc.tensor.matmul(
                pv[:],
                x_sb[:, 1, kc * 128 : (kc + 1) * 128],
                wq_sb[:, 1, C : 2 * C],
                start=False,
                stop=True,
            )
            nc.vector.tensor_tensor(
                out=vaugT[:, kc, :, 0:HD],
                in0=pv[:].rearrange("p (h d) -> p h d", h=NH),
                in1=bv_bc[:].rearrange("p (h d) -> p h d", h=NH),
                op=OP.add,
            )

        # ---- attention ----
        attn = sb.tile([128, 2, N], f32r, tag="attn")
        for h in range(NH):
            hp = 32 * h
            for qci, (q0, qw) in enumerate(QCS):
                pO = ps_mm.tile([HD + 1, 1024], f32, tag="mm")

                def score_stage(kc):
                    pS = ps_S.tile([128, 1024], f32, tag="S")
                    for s0 in range(0, qw, 512):
                        sw = min(512, qw - s0)
                        nc.tensor.matmul(
                            pS[:, s0 : s0 + sw],
                            k_pad[:, h, kc * 128 : (kc + 1) * 128],
                            q_all[:, q0 + s0 : q0 + s0 + sw],
                            start=True,
                            stop=True,
                        )
                    pT = pT_pool.tile([128, 1024], bf16, tag="pT")
                    nc.scalar.activation(
                        out=pT[:, 0:qw], in_=pS[:, 0:qw], func=AF.Exp, scale=SCALE
                    )
                    return pT

                def out_stage(kc, pT):
                    for s0 in range(0, qw, 512):
                        sw = min(512, qw - s0)
                        nc.tensor.matmul(
                            pO[:, s0 : s0 + sw],
                            vaugT[:, kc, h, :],
                            pT[:, s0 : s0 + sw],
                            start=(kc == 0),
                            stop=(kc == NKC - 1),
                        )

                # software pipeline: issue S^T(kc+1) before out(kc) so the PE
                # never idles waiting for exp(kc)
                pT_prev = score_stage(0)
                for kc in range(1, NKC):
                    pT_cur = score_stage(kc)
                    out_stage(kc - 1, pT_prev)
                    pT_prev = pT_cur
                out_stage(NKC - 1, pT_prev)

                # normalize: attn[d, q] = pO[d, q] / pO[64, q].  The softmax
                # denominator row is bounced through DRAM twice: once to
                # reshape [1, qw] -> [128, qw/128] so the reciprocal runs on
                # all 128 lanes, once to broadcast 1/l across 64 partitions.
                lbi = h * 3 + qci
                lrow = small.tile([1, 1024], f32, tag="lrow")
                nc.vector.tensor_copy(lrow[:, 0:qw], pO[HD : HD + 1, 0:qw])
                nc.sync.dma_start(out=lb_d[lbi : lbi + 1, 0:qw], in_=lrow[:, 0:qw])
                lpar = small.tile([128, 8], f32, tag="lpar")
                fw = qw // 128
                nc.sync.dma_start(
                    out=lpar[:, 0:fw],
                    in_=lb_d[lbi, 0:qw].rearrange("(p f) -> p f", p=128),
                )
                rpar = small.tile([128, 8], f32, tag="rpar")
                nc.vector.reciprocal(out=rpar[:, 0:fw], in_=lpar[:, 0:fw])
                nc.sync.dma_start(
                    out=rb_d[lbi, 0:qw].rearrange("(p f) -> p f", p=128),
                    in_=rpar[:, 0:fw],
                )
                linv_bc = small.tile([64, 1024], f32, tag="linv_bc")
                nc.sync.dma_start(
                    out=linv_bc[:, 0:qw],
                    in_=rb_d[lbi : lbi + 1, 0:qw].to_broadcast((64, qw)),
                )
                r0 = (h % 2) * 64
                nc.vector.tensor_tensor(
                    out=attn[r0 : r0 + 64, h // 2, q0 : q0 + qw],
                    in0=pO[0:HD, 0:qw],
                    in1=linv_bc[:, 0:qw],
                    op=OP.mult,
                )

        # ---- depthwise 3x3 conv on v_img (BN folded into dww/bpe) ----
        pe = sb.tile([128, 2, HW, HW], f32, tag="pe")
        v3 = v_img[:].rearrange("p a (y x) -> p a y x", y=HW)
        for a in range(2):
            # center tap (ky=1, kx=1) initializes
            nc.vector.tensor_scalar(
                pe[:, a], v3[:, a], dww_sb[:, a, 4:5], None, OP.mult
            )
            for ky in range(3):
                for kx in range(3):
                    if ky == 1 and kx == 1:
                        continue
                    dy, dx = ky - 1, kx - 1
                    y0, y1 = max(0, -dy), HW - max(0, dy)
                    x0, x1 = max(0, -dx), HW - max(0, dx)
                    t = ky * 3 + kx
                    nc.vector.scalar_tensor_tensor(
                        out=pe[:, a, y0:y1, x0:x1],
                        in0=v3[:, a, y0 + dy : y1 + dy, x0 + dx : x1 + dx],
                        scalar=dww_sb[:, a, t : t + 1],
                        in1=pe[:, a, y0:y1, x0:x1],
                        op0=OP.mult,
                        op1=OP.add,
                    )
            # attn += pe + bpe  (fused: (pe add bpe) add attn)
            nc.vector.scalar_tensor_tensor(
                out=attn[:, a, :],
                in0=pe[:, a].rearrange("p y x -> p (y x)"),
                scalar=bpe_sb[:, a : a + 1],
                in1=attn[:, a, :],
                op0=OP.add,
                op1=OP.add,
            )

        # ---- proj matmul ----
        out_sb = sb.tile([128, 2, N], f32, tag="out_sb")
        for oc in range(2):
            for n0, nw in NSUBS:
                pp = ps_mm.tile([128, 512], f32, tag="mm")
                nc.tensor.matmul(
                    pp[:, 0:nw],
                    wp_sb[:, 0, oc * 128 : (oc + 1) * 128],
                    attn[:, 0, n0 : n0 + nw],
                    start=True,
                    stop=False,
                )
                nc.tensor.matmul(
                    pp[:, 0:nw],
                    wp_sb[:, 1, oc * 128 : (oc + 1) * 128],
                    attn[:, 1, n0 : n0 + nw],
                    start=False,
                    stop=True,
                )
                nc.vector.tensor_scalar(
                    out_sb[:, oc, n0 : n0 + nw],
                    pp[:, 0:nw],
                    bproj_sb[:, oc : oc + 1],
                    None,
                    OP.add,
                )

        nc.sync.dma_start(
            out=out_d[:].rearrange("(a p) n -> p a n", p=128), in_=out_sb[:]
        )

    nc.finalize()
    return nc


def _prep_weights(qkv_w, qkv_g, qkv_b, qkv_m, qkv_v,
                  pe_w, pe_g, pe_b, pe_m, pe_v,
                  proj_w, proj_g, proj_b, proj_m, proj_v):
    """Fold BN into conv weights (host-side, numpy) and permute qkv output
    channels into [q_all | k_all | v0 | v1] blocks."""
    f = np.float32

    inv_q = (qkv_g / np.sqrt(qkv_v + BN_EPS)).astype(f)
    Wq = (qkv_w * inv_q[:, None]).astype(f)
    bq = (qkv_b - qkv_m * inv_q).astype(f)

    perm = []
    for h in range(NH):
        perm += list(range(h * 128, h * 128 + 32))          # q
    for h in range(NH):
        perm += list(range(h * 128 + 32, h * 128 + 64))     # k
    for h in range(NH):
        perm += list(range(h * 128 + 64, h * 128 + 128))    # v
    perm = np.array(perm)

    Wq_p = Wq[perm]            # [512, 256]
    bq_p = bq[perm]            # [512]
    wqkvT = np.ascontiguousarray(Wq_p.T)                    # [256, 512]
    bqkv = np.ascontiguousarray(bq_p.reshape(4, 128).T)     # [128, 4]
    bv = np.ascontiguousarray(bq_p[256:512].reshape(1, C))  # [1, 256]

    inv_pe = (pe_g / np.sqrt(pe_v + BN_EPS)).astype(f)
    dw = (pe_w[:, 0] * inv_pe[:, None, None]).astype(f).reshape(C, 9)
    dww = np.ascontiguousarray(dw.reshape(2, 128, 9).transpose(1, 0, 2))
    bpe = np.ascontiguousarray(
        (pe_b - pe_m * inv_pe).astype(f).reshape(2, 128).T
    )

    inv_p = (proj_g / np.sqrt(proj_v + BN_EPS)).astype(f)
    Wp = (proj_w * inv_p[:, None]).astype(f)
    bp = (proj_b - proj_m * inv_p).astype(f)
    wprojT = np.ascontiguousarray(Wp.T)                     # [256, 256]
    bproj = np.ascontiguousarray(bp.reshape(2, 128).T)      # [128, 2]

    return dict(wqkvT=wqkvT, bqkv=bqkv, bv=bv, wprojT=wprojT, bproj=bproj,
                dww=dww, bpe=bpe)


_NC_CACHE = None


def _get_module():
    global _NC_CACHE
    if _NC_CACHE is None:
        _NC_CACHE = _build_module()
    return _NC_CACHE


def kernel(x, qkv_w, qkv_g, qkv_b, qkv_m, qkv_v,
           pe_w, pe_g, pe_b, pe_m, pe_v,
           proj_w, proj_g, proj_b, proj_m, proj_v,
           _trace=False):
    from concourse.bass_utils import run_bass_kernel_spmd

    B = x.shape[0]
    assert x.shape == (B, C, HW, HW)
    w = _prep_weights(qkv_w, qkv_g, qkv_b, qkv_m, qkv_v,
                      pe_w, pe_g, pe_b, pe_m, pe_v,
                      proj_w, proj_g, proj_b, proj_m, proj_v)

    nc = _get_module()
    in_maps = []
    for b in range(B):
        m = {"x": np.ascontiguousarray(x[b].reshape(C, N), np.float32)}
        m.update(w)
        in_maps.append(m)

    res = run_bass_kernel_spmd(nc, in_maps, core_ids=list(range(B)),
                               trace=_trace)
    out = np.stack([res.results[b]["out"] for b in range(B)])
    out = out.reshape(B, C, HW, HW)
    if _trace:
        return out, res
    return out
